# revision 4
# baseline (speedup 1.0000x reference)
"""MoE layer (top-2 of 8 experts, d_model=2048, d_hid=4096) on 8 trn2 cores.

Strategy: expert-parallel with host-side token dispatch (the all-to-all
equivalent). Core e holds expert e's weights and processes only the tokens
routed to expert e (capacity-padded). The router math (logits -> top-2
combine weight) is recomputed on-device per core for its own tokens; the
host's numpy routing is used only to PLACE tokens on cores. FFN matmuls run
as float32r (full PE rate); the router matmul runs fp32 so the on-device
top-2 selection agrees with the host placement.

Per-core device pipeline (tokens chunked by <=512 along the free dim):
  router: psum[128tok, 8] = sum_k x32[k,tok].T @ rw[k, 8]   (fp32)
          s = exp(l_e - m1) / (1 + exp(m2 - m1))            (DVE/ACT)
  L1:     h'[h, tok] = gelu(sum_k w1[k,h].T @ x[k,tok] + b1)  (f32r, ACT)
  L2:     y[tok, d] = (sum_h h'[h,tok].T @ w2[h, d] + b2) * s  (f32r, DVE)
"""
import sys
import os

sys.path.insert(0, "/opt/trn_rl_repo")

import numpy as np

import concourse.bass as bass
import concourse.tile as tile
from concourse import bacc, mybir
from concourse.bass_utils import run_bass_kernel_spmd

P = 128
D_MODEL = 2048
D_HID = 4096
N_EXP = 8
TOP_K = 2
F32R = mybir.dt.float32r
F32 = mybir.dt.float32
KT1 = D_MODEL // P   # 16 k-tiles in layer 1
KT2 = D_HID // P     # 32 k-tiles in layer 2
HT = D_HID // P      # 32 h-tiles of layer-1 output
DT = D_MODEL // 256  # 8 d-tiles of 256 in layer 2


def _chunks_of(C):
    """Split C (multiple of 128) into matmul-friendly chunks (256..512)."""
    assert C % P == 0 and C >= 256
    out = []
    rem = C
    while rem > 640:
        out.append(512)
        rem -= 512
    if rem == 640:
        out.extend([384, 256])
    else:
        out.append(rem)  # 256..512 (512/384/256)
    assert sum(out) == C and all(256 <= c <= 512 for c in out)
    return out


def build_moe(C):
    chunks = _chunks_of(C)
    nt_total = C // P
    nc = bacc.Bacc("TRN2", target_bir_lowering=False, debug=False)

    xTr = nc.dram_tensor("xTr", [D_MODEL, C], F32R, kind="ExternalInput").ap()
    xT32 = nc.dram_tensor("xT32", [D_MODEL, C], F32, kind="ExternalInput").ap()
    w1 = nc.dram_tensor("w1", [D_MODEL, D_HID], F32R, kind="ExternalInput").ap()
    b1 = nc.dram_tensor("b1", [D_HID], F32, kind="ExternalInput").ap()
    w2 = nc.dram_tensor("w2", [D_HID, D_MODEL], F32R, kind="ExternalInput").ap()
    b2 = nc.dram_tensor("b2", [D_MODEL], F32, kind="ExternalInput").ap()
    rw = nc.dram_tensor("rw", [D_MODEL, N_EXP], F32, kind="ExternalInput").ap()
    oh = nc.dram_tensor("oh", [P, N_EXP], F32, kind="ExternalInput").ap()
    y = nc.dram_tensor("y", [C, D_MODEL], F32, kind="ExternalOutput").ap()

    with tile.TileContext(nc) as tc:
        with (
            tc.tile_pool(name="singles", bufs=1) as singles,
            tc.tile_pool(name="xpool", bufs=1) as xpool,
            tc.tile_pool(name="x32pool", bufs=3) as x32pool,
            tc.tile_pool(name="hpool", bufs=1) as hpool,
            tc.tile_pool(name="w1pool", bufs=2) as w1pool,
            tc.tile_pool(name="w2pool", bufs=2) as w2pool,
            tc.tile_pool(name="ypool", bufs=3) as ypool,
            tc.tile_pool(name="rpool", bufs=2) as rpool,
            tc.tile_pool(name="psr", bufs=1, space="PSUM") as psr,
            tc.tile_pool(name="ps1", bufs=3, space="PSUM") as ps1,
            tc.tile_pool(name="ps2", bufs=4, space="PSUM") as ps2,
        ):
            # ---- constants ----
            rw_sb = singles.tile([P, KT1, N_EXP], F32)
            for kt in range(KT1):
                nc.sync.dma_start(out=rw_sb[:, kt, :], in_=rw[kt * P:(kt + 1) * P, :])
            oh_sb = singles.tile([P, N_EXP], F32)
            nc.sync.dma_start(out=oh_sb, in_=oh)
            # b1 as per-partition columns: b1_sb[:, ht] = b1[ht*128:(ht+1)*128]
            b1v = b1.rearrange("(a p) -> p a", p=P)  # [128, HT]
            b1_sb = singles.tile([P, HT], F32)
            nc.sync.dma_start(out=b1_sb, in_=b1v)
            # b2 broadcast across partitions: [128, D_MODEL]
            b2_sb = singles.tile([P, D_MODEL], F32)
            b2_bc = bass.AP(tensor=b2.tensor, offset=b2.offset,
                            ap=[[0, P]] + list(b2.ap))
            nc.sync.dma_start(out=b2_sb, in_=b2_bc)
            # per-token combine weight, written column by column
            s_all = singles.tile([P, nt_total], F32)

            tok0 = 0
            for cs in chunks:
                ntt = cs // P
                # ---- load x chunk (f32r) ----
                xr = []
                for kt in range(KT1):
                    t = xpool.tile([P, 512], F32R, tag=f"x{kt}")
                    nc.sync.dma_start(
                        out=t[:, :cs],
                        in_=xTr[kt * P:(kt + 1) * P, tok0:tok0 + cs])
                    xr.append(t)

                # ---- router for this chunk's token tiles ----
                for ti in range(ntt):
                    g = tok0 // P + ti
                    pr = psr.tile([P, N_EXP], F32, tag="pr")
                    for kt in range(KT1):
                        x32t = x32pool.tile([P, P], F32, tag="x32")
                        nc.sync.dma_start(
                            out=x32t,
                            in_=xT32[kt * P:(kt + 1) * P,
                                     tok0 + ti * P:tok0 + (ti + 1) * P])
                        nc.tensor.matmul(pr[:], lhsT=x32t[:], rhs=rw_sb[:, kt, :],
                                         start=(kt == 0), stop=(kt == KT1 - 1))
                    lg = rpool.tile([P, N_EXP], F32, tag="lg")
                    nc.vector.tensor_copy(lg[:], pr[:])
                    m1 = rpool.tile([P, 1], F32, tag="m1")
                    nc.vector.reduce_max(out=m1[:], in_=lg[:], axis=mybir.AxisListType.X)
                    d8 = rpool.tile([P, N_EXP], F32, tag="d8")
                    nc.vector.tensor_scalar(d8[:], lg[:], m1[:], None,
                                            op0=mybir.AluOpType.subtract)
                    e1 = rpool.tile([P, N_EXP], F32, tag="e1")
                    nc.scalar.activation(e1[:], d8[:], mybir.ActivationFunctionType.Exp)
                    ge = rpool.tile([P, N_EXP], F32, tag="ge")
                    nc.vector.tensor_scalar(ge[:], lg[:], m1[:], None,
                                            op0=mybir.AluOpType.is_ge)
                    mk = rpool.tile([P, N_EXP], F32, tag="mk")
                    nc.vector.tensor_sub(mk[:], e1[:], ge[:])
                    m2 = rpool.tile([P, 1], F32, tag="m2")
                    nc.vector.reduce_max(out=m2[:], in_=mk[:], axis=mybir.AxisListType.X)
                    dn = rpool.tile([P, 1], F32, tag="dn")
                    nc.vector.tensor_scalar_add(dn[:], m2[:], 1.0)
                    rc = rpool.tile([P, 1], F32, tag="rc")
                    nc.vector.reciprocal(out=rc[:], in_=dn[:])
                    me = rpool.tile([P, N_EXP], F32, tag="me")
                    nc.vector.tensor_mul(me[:], e1[:], oh_sb[:])
                    ms = rpool.tile([P, 1], F32, tag="ms")
                    nc.vector.reduce_sum(out=ms[:], in_=me[:], axis=mybir.AxisListType.X)
                    nc.vector.tensor_mul(s_all[:, g:g + 1], ms[:], rc[:])

                # ---- layer 1: h'[h, tok] = gelu(w1.T @ x + b1), f32r out ----
                htiles = []
                for ht in range(HT):
                    w1t = []
                    for kt in range(KT1):
                        t = w1pool.tile([P, P], F32R, tag=f"w1_{kt}")
                        nc.sync.dma_start(
                            out=t,
                            in_=w1[kt * P:(kt + 1) * P, ht * P:(ht + 1) * P])
                        w1t.append(t)
                    p1 = ps1.tile([P, 512], F32, tag="p1")
                    for kt in range(KT1):
                        nc.tensor.matmul(p1[:, :cs], lhsT=w1t[kt][:], rhs=xr[kt][:, :cs],
                                         start=(kt == 0), stop=(kt == KT1 - 1))
                    h_t = hpool.tile([P, 512], F32R, tag=f"h{ht}")
                    nc.scalar.activation(h_t[:, :cs], p1[:, :cs],
                                         mybir.ActivationFunctionType.Gelu,
                                         bias=b1_sb[:, ht:ht + 1])
                    htiles.append(h_t)

                # ---- layer 2: y[tok, d] = (h'.T @ w2 + b2) * s ----
                for dt in range(DT):
                    w2t = []
                    for kt in range(KT2):
                        t = w2pool.tile([P, 256], F32R, tag=f"w2_{kt}")
                        nc.sync.dma_start(
                            out=t,
                            in_=w2[kt * P:(kt + 1) * P, dt * 256:(dt + 1) * 256])
                        w2t.append(t)
                    for ti in range(ntt):
                        g = tok0 // P + ti
                        p2 = ps2.tile([P, 256], F32, tag="p2")
                        for kt in range(KT2):
                            nc.tensor.matmul(p2[:],
                                             lhsT=htiles[kt][:, ti * P:(ti + 1) * P],
                                             rhs=w2t[kt][:],
                                             start=(kt == 0), stop=(kt == KT2 - 1))
                        yt = ypool.tile([P, 256], F32, tag="y")
                        nc.vector.tensor_add(yt[:], p2[:], b2_sb[:, dt * 256:(dt + 1) * 256])
                        nc.vector.tensor_scalar(yt[:], yt[:], s_all[:, g:g + 1], None,
                                                op0=mybir.AluOpType.mult)
                        nc.sync.dma_start(
                            out=y[tok0 + ti * P:tok0 + (ti + 1) * P,
                                  dt * 256:(dt + 1) * 256],
                            in_=yt[:])
                tok0 += cs
    nc.compile()
    return nc


def build_null(C):
    """Null kernel with identical I/O signature — for dispatch-overhead calibration."""
    nc = bacc.Bacc("TRN2", target_bir_lowering=False, debug=False)
    xTr = nc.dram_tensor("xTr", [D_MODEL, C], F32R, kind="ExternalInput").ap()
    nc.dram_tensor("xT32", [D_MODEL, C], F32, kind="ExternalInput").ap()
    nc.dram_tensor("w1", [D_MODEL, D_HID], F32R, kind="ExternalInput").ap()
    nc.dram_tensor("b1", [D_HID], F32, kind="ExternalInput").ap()
    nc.dram_tensor("w2", [D_HID, D_MODEL], F32R, kind="ExternalInput").ap()
    nc.dram_tensor("b2", [D_MODEL], F32, kind="ExternalInput").ap()
    nc.dram_tensor("rw", [D_MODEL, N_EXP], F32, kind="ExternalInput").ap()
    nc.dram_tensor("oh", [P, N_EXP], F32, kind="ExternalInput").ap()
    y = nc.dram_tensor("y", [C, D_MODEL], F32, kind="ExternalOutput").ap()
    with tile.TileContext(nc) as tc:
        with tc.tile_pool(name="sbuf", bufs=1) as pool:
            t = pool.tile([P, 256], F32)
            nc.sync.dma_start(out=t, in_=xTr[0:P, 0:256].bitcast(F32))
            nc.sync.dma_start(out=y[0:P, 0:256], in_=t[:])
    nc.compile()
    return nc


def _route_host(xt, router_w):
    """numpy top-2 routing (placement only; weights recomputed on device)."""
    logits = xt @ router_w                       # [T, E] fp32
    i1 = np.argmax(logits, axis=1)
    masked = logits.copy()
    masked[np.arange(xt.shape[0]), i1] = -np.inf
    i2 = np.argmax(masked, axis=1)
    return i1, i2


def _build_in_maps(xt, inputs, idx, cnts, C):
    D = xt.shape[1]
    router_w = np.ascontiguousarray(inputs["router_w"], dtype=np.float32)
    in_maps = []
    for e in range(N_EXP):
        xe = np.zeros((D, C), dtype=np.float32)
        xe[:, :cnts[e]] = xt[idx[e]].T
        ohe = np.zeros((P, N_EXP), dtype=np.float32)
        ohe[:, e] = 1.0
        in_maps.append({
            "xTr": xe, "xT32": xe,
            "w1": np.ascontiguousarray(inputs["w1"][e], dtype=np.float32),
            "b1": np.ascontiguousarray(inputs["b1"][e], dtype=np.float32),
            "w2": np.ascontiguousarray(inputs["w2"][e], dtype=np.float32),
            "b2": np.ascontiguousarray(inputs["b2"][e], dtype=np.float32),
            "rw": router_w, "oh": ohe,
        })
    return in_maps


_NC_CACHE = {}


def _get_nc(C):
    if C not in _NC_CACHE:
        _NC_CACHE[C] = build_moe(C)
    return _NC_CACHE[C]


def kernel(x, router_w, w1, b1, w2, b2):
    x = np.asarray(x, dtype=np.float32)
    router_w = np.asarray(router_w, dtype=np.float32)
    w1 = np.asarray(w1, dtype=np.float32)
    b1 = np.asarray(b1, dtype=np.float32)
    w2 = np.asarray(w2, dtype=np.float32)
    b2 = np.asarray(b2, dtype=np.float32)

    Bc, Sc, D = x.shape
    T = Bc * Sc
    xt = np.ascontiguousarray(x.reshape(T, D))

    i1, i2 = _route_host(xt, router_w)
    idx = [np.where((i1 == e) | (i2 == e))[0] for e in range(N_EXP)]
    cnts = [len(ix) for ix in idx]
    C = max(512, -(-max(cnts) // P) * P)

    nc = _get_nc(C)

    in_maps = _build_in_maps(xt, {"router_w": router_w, "w1": w1, "b1": b1,
                                  "w2": w2, "b2": b2}, idx, cnts, C)

    res = run_bass_kernel_spmd(nc, in_maps, core_ids=list(range(N_EXP)))

    out = np.zeros((T, D), dtype=np.float32)
    for e in range(N_EXP):
        ye = res.results[e]["y"]
        out[idx[e]] += ye[:cnts[e]]
    return out.reshape(Bc, Sc, D)


# revision 12
# speedup vs baseline: 1.1558x; 1.1558x over previous
"""MoE layer (top-2 of 8 experts, d_model=2048, d_hid=4096) on 8 trn2 cores.

Strategy: expert-parallel with host-side token dispatch (the all-to-all
equivalent). Core e holds expert e's weights and processes only the tokens
routed to expert e (capacity-padded to C, a multiple of 128). The router
math (logits -> top-2 combine weight) is recomputed on-device per core for
its own tokens; the host's numpy routing is used only to PLACE tokens.
Near-tie top-2 flips between host (fp32) and device (bf16/f32r) routing are
harmless: the combine weight w = p_e / (p_top1 + p_top2) is symmetric in the
top-2 set and continuous in the logits, so a flip at a near-tie perturbs the
output by only the logit-noise magnitude.

Per-core device pipeline:
  router: psum[128tok, 8] = sum_k x[k,tok].T @ rw[k, 8]       (PE)
          s = exp(l_e - m1) / (1 + exp(m2 - m1))              (DVE/ACT)
  L1:     h'[h, tok] = gelu(sum_k w1[k,h].T @ x[k,tok] + b1)  (PE + ACT)
  L2:     y[tok, d] = (sum_h h'[h,tok].T @ w2[h, d] + b2) * s (PE + DVE)

Two variants:
  - bf16 "resident" (default): x and h' stay in SBUF for the whole token
    range; w1/w2 stream from HBM exactly once (~40MB/core) -> compute-bound.
  - f32r "chunked" (MOE_DTYPE=f32r): TF32-class precision (~2e-4 rel err),
    tokens processed in <=512 chunks, weights re-streamed per chunk.
"""
import os
import sys

sys.path.insert(0, "/opt/trn_rl_repo")

import numpy as np
import ml_dtypes

import concourse.bass as bass
import concourse.tile as tile
from concourse import bacc, mybir
from concourse.bass_utils import run_bass_kernel_spmd
from concourse.masks import make_identity

P = 128
D_MODEL = 2048
D_HID = 4096
N_EXP = 8
F32R = mybir.dt.float32r
F32 = mybir.dt.float32
BF16 = mybir.dt.bfloat16
KT1 = D_MODEL // P   # 16 k-tiles in layer 1
KT2 = D_HID // P     # 32 k-tiles in layer 2
HT = D_HID // P      # 32 h-tiles of layer-1 output
DT = D_MODEL // 256  # 8 d-tiles of 256 in layer 2

WDT = BF16 if os.environ.get("MOE_DTYPE", "bf16") == "bf16" else F32R
# largest C whose x + h' residency fits SBUF in bf16
C_RESIDENT_MAX = 1408


def _spans_of(C):
    """Split C (multiple of 128, >=512) into matmul-friendly spans (256..512)."""
    assert C % P == 0 and C >= 512
    out = []
    rem = C
    while rem > 640:
        out.append(512)
        rem -= 512
    if rem == 640:
        out.extend([384, 256])
    else:
        out.append(rem)
    assert sum(out) == C and all(256 <= c <= 512 for c in out)
    return [(sum(out[:i]), c) for i, c in enumerate(out)]


def _declare_io(nc, C, wdt):
    t = {}
    t["xTw"] = nc.dram_tensor("xTw", [D_MODEL, C], wdt, kind="ExternalInput").ap()
    t["w1"] = nc.dram_tensor("w1", [D_MODEL, D_HID], wdt, kind="ExternalInput").ap()
    t["b1"] = nc.dram_tensor("b1", [D_HID], F32, kind="ExternalInput").ap()
    t["w2"] = nc.dram_tensor("w2", [D_HID, D_MODEL], wdt, kind="ExternalInput").ap()
    t["b2"] = nc.dram_tensor("b2", [D_MODEL], F32, kind="ExternalInput").ap()
    t["rw"] = nc.dram_tensor("rw", [D_MODEL, N_EXP], wdt, kind="ExternalInput").ap()
    t["oh"] = nc.dram_tensor("oh", [P, N_EXP], F32, kind="ExternalInput").ap()
    t["y"] = nc.dram_tensor("y", [C, D_MODEL], F32, kind="ExternalOutput").ap()
    return t


def _load_consts(nc, singles, io, wdt):
    rw_sb = singles.tile([P, KT1, N_EXP], wdt)
    rwv = io["rw"].rearrange("(kt p) e -> p kt e", p=P)
    nc.sync.dma_start(out=rw_sb, in_=rwv)
    oh_sb = singles.tile([P, N_EXP], F32)
    nc.sync.dma_start(out=oh_sb, in_=io["oh"])
    b1v = io["b1"].rearrange("(a p) -> p a", p=P)  # [128, HT]
    b1_sb = singles.tile([P, HT], F32)
    nc.sync.dma_start(out=b1_sb, in_=b1v)
    b2_sb = singles.tile([P, D_MODEL], F32)
    b2_bc = bass.AP(tensor=io["b2"].tensor, offset=io["b2"].offset,
                    ap=[[0, P]] + list(io["b2"].ap))
    nc.sync.dma_start(out=b2_sb, in_=b2_bc)
    return rw_sb, oh_sb, b1_sb, b2_sb


def _router_block(nc, pools, xr, rw_sb, oh_sb, s_all, ident, spans, g0):
    """Combine weights for a block of token tiles in one batched chain.

    Matmul with rw stationary (8-col LDWEIGHTS) -> logitsT [8, tok] psum;
    PE-transpose each 128-token tile into a [128, ntt*8] block; then one
    ~12-op DVE/ACT chain computes s = exp(l_e - m1)/(1 + exp(m2 - m1)) for
    all tiles at once into s_all[:, g0:g0+ntt].
    """
    rpool, psr = pools
    C_blk = sum(cs for _, cs in spans)
    ntt = C_blk // P
    lgT_sb = rpool.tile([8, C_blk], F32, tag="lgT")
    for off, cs in spans:
        lgT_ps = psr.tile([8, 512], F32, tag="lgT_ps")
        for kt in range(KT1):
            nc.tensor.matmul(lgT_ps[:, :cs], lhsT=rw_sb[:, kt, :],
                             rhs=xr[kt][:, off:off + cs],
                             start=(kt == 0), stop=(kt == KT1 - 1))
        nc.vector.tensor_copy(lgT_sb[:, off:off + cs], lgT_ps[:, :cs])
    pr_all = psr.tile([P, ntt * N_EXP], F32, tag="pr_all")
    for t in range(ntt):
        nc.tensor.transpose(pr_all[:, t * N_EXP:(t + 1) * N_EXP],
                            lgT_sb[:, t * P:(t + 1) * P], ident[0:N_EXP, 0:N_EXP])
    lg = rpool.tile([P, ntt, N_EXP], F32, tag="lg")
    nc.vector.tensor_copy(lg[:], pr_all[:].rearrange("p (t e) -> p t e", e=N_EXP))
    m1 = rpool.tile([P, ntt, 1], F32, tag="m1")
    nc.vector.reduce_max(out=m1[:], in_=lg[:], axis=mybir.AxisListType.X)
    m1b = m1[:, :, 0:1].to_broadcast([P, ntt, N_EXP])
    d8 = rpool.tile([P, ntt, N_EXP], F32, tag="d8")
    nc.vector.tensor_tensor(d8[:], lg[:], m1b, mybir.AluOpType.subtract)
    e1 = rpool.tile([P, ntt, N_EXP], F32, tag="e1")
    nc.scalar.activation(e1[:], d8[:], mybir.ActivationFunctionType.Exp)
    ge = rpool.tile([P, ntt, N_EXP], F32, tag="ge")
    nc.vector.tensor_tensor(ge[:], lg[:], m1b, mybir.AluOpType.is_ge)
    mk = rpool.tile([P, ntt, N_EXP], F32, tag="mk")
    nc.vector.tensor_sub(mk[:], e1[:], ge[:])
    m2 = rpool.tile([P, ntt, 1], F32, tag="m2")
    nc.vector.reduce_max(out=m2[:], in_=mk[:], axis=mybir.AxisListType.X)
    dn = rpool.tile([P, ntt, 1], F32, tag="dn")
    nc.vector.tensor_scalar_add(dn[:], m2[:], 1.0)
    rc = rpool.tile([P, ntt, 1], F32, tag="rc")
    nc.vector.reciprocal(out=rc[:], in_=dn[:])
    ohb = oh_sb[:].rearrange("p (o e) -> p o e", o=1).to_broadcast([P, ntt, N_EXP])
    me = rpool.tile([P, ntt, N_EXP], F32, tag="me")
    nc.vector.tensor_tensor(me[:], e1[:], ohb, mybir.AluOpType.mult)
    ms = rpool.tile([P, ntt, 1], F32, tag="ms")
    nc.vector.reduce_sum(out=ms[:], in_=me[:], axis=mybir.AxisListType.X)
    nc.vector.tensor_mul(s_all[:, g0:g0 + ntt], ms[:, :, 0], rc[:, :, 0])


def build_moe_resident(C, wdt=BF16, reps=1, ablate=()):
    """x and h' SBUF-resident for all C tokens; weights stream exactly once.

    reps>1 wraps the whole body in a hardware loop (timing use only)."""
    spans = _spans_of(C)
    nt = C // P
    nc = bacc.Bacc("TRN2", target_bir_lowering=False, debug=False)
    io = _declare_io(nc, C, wdt)
    from contextlib import nullcontext

    with tile.TileContext(nc) as tc:
        with (
            tc.tile_pool(name="singles", bufs=1) as singles,
            tc.tile_pool(name="xpool", bufs=1) as xpool,
            tc.tile_pool(name="hpool", bufs=1) as hpool,
            tc.tile_pool(name="w1pool", bufs=2) as w1pool,
            tc.tile_pool(name="w2pool", bufs=2) as w2pool,
            tc.tile_pool(name="ypool", bufs=4) as ypool,
            tc.tile_pool(name="rpool", bufs=2) as rpool,
            tc.tile_pool(name="psr", bufs=1, space="PSUM") as psr,
            tc.tile_pool(name="ps1", bufs=3, space="PSUM") as ps1,
            tc.tile_pool(name="ps2", bufs=3, space="PSUM") as ps2,
            tc.For_i(0, reps, 1) if reps > 1 else nullcontext(),
        ):
            rw_sb, oh_sb, b1_sb, b2_sb = _load_consts(nc, singles, io, wdt)
            s_all = singles.tile([P, nt], F32)
            ident = singles.tile([P, P], F32, tag="ident")
            make_identity(nc, ident)

            xr = []
            for kt in range(KT1):
                t = xpool.tile([P, C], wdt, tag=f"x{kt}")
                nc.sync.dma_start(out=t, in_=io["xTw"][kt * P:(kt + 1) * P, :])
                xr.append(t)

            if "router" in ablate:
                nc.vector.memset(s_all[:], 1.0)
            else:
                _router_block(nc, (rpool, psr), xr, rw_sb, oh_sb, s_all, ident,
                              spans, 0)

            # layer 1: h'[h, tok] = gelu(w1.T @ x + b1)
            htiles = []
            if "l1" in ablate:
                for ht in range(HT):
                    h_t = hpool.tile([P, C], wdt, tag=f"h{ht}")
                    nc.vector.memset(h_t[:], 0.01)
                    htiles.append(h_t)
            w1v = io["w1"].rearrange("(kt p) h -> p kt h", p=P)  # [128, KT1, D_HID]
            for ht in range(HT if "l1" not in ablate else 0):
                w1t = w1pool.tile([P, KT1, P], wdt, tag="w1")
                nc.sync.dma_start(out=w1t, in_=w1v[:, :, ht * P:(ht + 1) * P])
                h_t = hpool.tile([P, C], wdt, tag=f"h{ht}")
                for off, cs in spans:
                    p1 = ps1.tile([P, 512], F32, tag="p1")
                    for kt in range(KT1):
                        nc.tensor.matmul(p1[:, :cs], lhsT=w1t[:, kt, :],
                                         rhs=xr[kt][:, off:off + cs],
                                         start=(kt == 0), stop=(kt == KT1 - 1))
                    nc.scalar.activation(h_t[:, off:off + cs], p1[:, :cs],
                                         mybir.ActivationFunctionType.Gelu,
                                         bias=b1_sb[:, ht:ht + 1])
                htiles.append(h_t)

            # layer 2: y[tok, d] = (h'.T @ w2 + b2) * s
            w2v = io["w2"].rearrange("(kt p) d -> p kt d", p=P)  # [128, KT2, D_MODEL]
            for dt in range(DT if "l2" not in ablate else 0):
                w2t = w2pool.tile([P, KT2, 256], wdt, tag="w2")
                nc.sync.dma_start(out=w2t, in_=w2v[:, :, dt * 256:(dt + 1) * 256])
                for ti in range(nt):
                    p2 = ps2.tile([P, 256], F32, tag="p2")
                    for kt in range(KT2):
                        nc.tensor.matmul(p2[:],
                                         lhsT=htiles[kt][:, ti * P:(ti + 1) * P],
                                         rhs=w2t[:, kt, :],
                                         start=(kt == 0), stop=(kt == KT2 - 1))
                    yt = ypool.tile([P, 256], F32, tag="y")
                    nc.vector.tensor_add(yt[:], p2[:], b2_sb[:, dt * 256:(dt + 1) * 256])
                    nc.vector.tensor_scalar(yt[:], yt[:], s_all[:, ti:ti + 1], None,
                                            op0=mybir.AluOpType.mult)
                    nc.sync.dma_start(
                        out=io["y"][ti * P:(ti + 1) * P, dt * 256:(dt + 1) * 256],
                        in_=yt[:])
    nc.compile()
    return nc


def build_moe_chunked(C, wdt=F32R, reps=1):
    """Tokens processed in <=512 chunks; weights re-streamed per chunk."""
    spans = _spans_of(C)
    nt = C // P
    nc = bacc.Bacc("TRN2", target_bir_lowering=False, debug=False)
    io = _declare_io(nc, C, wdt)
    from contextlib import nullcontext

    with tile.TileContext(nc) as tc:
        with (
            tc.tile_pool(name="singles", bufs=1) as singles,
            tc.tile_pool(name="xpool", bufs=1) as xpool,
            tc.tile_pool(name="hpool", bufs=1) as hpool,
            tc.tile_pool(name="w1pool", bufs=2) as w1pool,
            tc.tile_pool(name="w2pool", bufs=2) as w2pool,
            tc.tile_pool(name="ypool", bufs=3) as ypool,
            tc.tile_pool(name="rpool", bufs=2) as rpool,
            tc.tile_pool(name="psr", bufs=1, space="PSUM") as psr,
            tc.tile_pool(name="ps1", bufs=3, space="PSUM") as ps1,
            tc.tile_pool(name="ps2", bufs=3, space="PSUM") as ps2,
            tc.For_i(0, reps, 1) if reps > 1 else nullcontext(),
        ):
            rw_sb, oh_sb, b1_sb, b2_sb = _load_consts(nc, singles, io, wdt)
            s_all = singles.tile([P, nt], F32)
            ident = singles.tile([P, P], F32, tag="ident")
            make_identity(nc, ident)
            w1v = io["w1"].rearrange("(kt p) h -> p kt h", p=P)
            w2v = io["w2"].rearrange("(kt p) d -> p kt d", p=P)

            for tok0, cs in spans:
                ntt = cs // P
                xr = []
                for kt in range(KT1):
                    t = xpool.tile([P, 512], wdt, tag=f"x{kt}")
                    nc.sync.dma_start(
                        out=t[:, :cs], in_=io["xTw"][kt * P:(kt + 1) * P, tok0:tok0 + cs])
                    xr.append(t)

                _router_block(nc, (rpool, psr), xr, rw_sb, oh_sb, s_all, ident,
                              [(0, cs)], tok0 // P)

                htiles = []
                for ht in range(HT):
                    w1t = w1pool.tile([P, KT1, P], wdt, tag="w1")
                    nc.sync.dma_start(out=w1t, in_=w1v[:, :, ht * P:(ht + 1) * P])
                    p1 = ps1.tile([P, 512], F32, tag="p1")
                    for kt in range(KT1):
                        nc.tensor.matmul(p1[:, :cs], lhsT=w1t[:, kt, :], rhs=xr[kt][:, :cs],
                                         start=(kt == 0), stop=(kt == KT1 - 1))
                    h_t = hpool.tile([P, 512], wdt, tag=f"h{ht}")
                    nc.scalar.activation(h_t[:, :cs], p1[:, :cs],
                                         mybir.ActivationFunctionType.Gelu,
                                         bias=b1_sb[:, ht:ht + 1])
                    htiles.append(h_t)

                for dt in range(DT):
                    w2t = w2pool.tile([P, KT2, 256], wdt, tag="w2")
                    nc.sync.dma_start(out=w2t, in_=w2v[:, :, dt * 256:(dt + 1) * 256])
                    for ti in range(ntt):
                        g = tok0 // P + ti
                        p2 = ps2.tile([P, 256], F32, tag="p2")
                        for kt in range(KT2):
                            nc.tensor.matmul(p2[:],
                                             lhsT=htiles[kt][:, ti * P:(ti + 1) * P],
                                             rhs=w2t[:, kt, :],
                                             start=(kt == 0), stop=(kt == KT2 - 1))
                        yt = ypool.tile([P, 256], F32, tag="y")
                        nc.vector.tensor_add(yt[:], p2[:],
                                             b2_sb[:, dt * 256:(dt + 1) * 256])
                        nc.vector.tensor_scalar(yt[:], yt[:], s_all[:, g:g + 1], None,
                                                op0=mybir.AluOpType.mult)
                        nc.sync.dma_start(
                            out=io["y"][tok0 + ti * P:tok0 + (ti + 1) * P,
                                        dt * 256:(dt + 1) * 256],
                            in_=yt[:])
    nc.compile()
    return nc


def build_moe(C, wdt=None, reps=1):
    wdt = WDT if wdt is None else wdt
    if wdt == BF16 and C <= C_RESIDENT_MAX:
        return build_moe_resident(C, wdt, reps=reps)
    return build_moe_chunked(C, wdt, reps=reps)


def build_null(C, wdt=None):
    """Null kernel with identical I/O signature — dispatch-overhead calibration."""
    wdt = WDT if wdt is None else wdt
    nc = bacc.Bacc("TRN2", target_bir_lowering=False, debug=False)
    io = _declare_io(nc, C, wdt)
    with tile.TileContext(nc) as tc:
        with tc.tile_pool(name="sbuf", bufs=1) as pool:
            t = pool.tile([P, 256], F32)
            nc.sync.dma_start(out=t, in_=io["b2"][0:256].rearrange("(a b) -> a b", a=1)
                              .broadcast(0, P))
            nc.sync.dma_start(out=io["y"][0:P, 0:256], in_=t[:])
    nc.compile()
    return nc


def _route_host(xt, router_w):
    """numpy top-2 routing (placement only; weights recomputed on device)."""
    logits = xt @ router_w
    i1 = np.argmax(logits, axis=1)
    masked = logits.copy()
    masked[np.arange(xt.shape[0]), i1] = -np.inf
    i2 = np.argmax(masked, axis=1)
    return i1, i2


def _build_in_maps(xt, inputs, idx, cnts, C, wdt=None):
    wdt = WDT if wdt is None else wdt
    np_w = ml_dtypes.bfloat16 if wdt == BF16 else np.float32
    D = xt.shape[1]
    in_maps = []
    for e in range(N_EXP):
        xe = np.zeros((D, C), dtype=np.float32)
        xe[:, :cnts[e]] = xt[idx[e]].T
        ohe = np.zeros((P, N_EXP), dtype=np.float32)
        ohe[:, e] = 1.0
        in_maps.append({
            "xTw": xe.astype(np_w),
            "w1": np.ascontiguousarray(inputs["w1"][e]).astype(np_w),
            "b1": np.ascontiguousarray(inputs["b1"][e], dtype=np.float32),
            "w2": np.ascontiguousarray(inputs["w2"][e]).astype(np_w),
            "b2": np.ascontiguousarray(inputs["b2"][e], dtype=np.float32),
            "rw": np.ascontiguousarray(inputs["router_w"]).astype(np_w),
            "oh": ohe,
        })
    return in_maps


_NC_CACHE = {}


def _get_nc(C):
    if C not in _NC_CACHE:
        _NC_CACHE[C] = build_moe(C)
    return _NC_CACHE[C]


def kernel(x, router_w, w1, b1, w2, b2):
    x = np.asarray(x, dtype=np.float32)
    inputs = {"router_w": np.asarray(router_w, dtype=np.float32),
              "w1": np.asarray(w1, dtype=np.float32),
              "b1": np.asarray(b1, dtype=np.float32),
              "w2": np.asarray(w2, dtype=np.float32),
              "b2": np.asarray(b2, dtype=np.float32)}

    Bc, Sc, D = x.shape
    T = Bc * Sc
    xt = np.ascontiguousarray(x.reshape(T, D))

    i1, i2 = _route_host(xt, inputs["router_w"])
    idx = [np.where((i1 == e) | (i2 == e))[0] for e in range(N_EXP)]
    cnts = [len(ix) for ix in idx]
    C = max(512, -(-max(cnts) // P) * P)

    nc = _get_nc(C)
    in_maps = _build_in_maps(xt, inputs, idx, cnts, C)
    res = run_bass_kernel_spmd(nc, in_maps, core_ids=list(range(N_EXP)))

    out = np.zeros((T, D), dtype=np.float32)
    for e in range(N_EXP):
        ye = res.results[e]["y"]
        out[idx[e]] += ye[:cnts[e]]
    return out.reshape(Bc, Sc, D)
